# revision 60
# baseline (speedup 1.0000x reference)
"""BiRNN encoder-decoder Trainium2 kernel, feature-major layout.

Data-parallel over batch (8 cores x 16 rows). All state is kept
feature-major: h lives in SBUF as [128 (H-chunk), 16 (batch)] fp16 columns,
weights are the PE stationary operand ([k-chunk, n-chunk] tiles of W.T) and
the state is the moving operand, so each recurrent matmul's cost scales with
the 16-wide batch (free size) instead of the 512-wide hidden dim. No
transposes anywhere: the PSUM output [128n, 16b] of one step is exactly the
moving layout the next step needs; tanh evacuates PSUM->SBUF directly.

Decoder feedback is algebraically folded into the layer-0 matmul: with
o0 = lin.h3 + lb and nxt = [o0, x0-o0, x1-x0+o0], layer-0's next-step input
projection W0.nxt becomes A.h3 + B2.[x0;x1] + c0 with A = W0.N.lin (rank-1,
precomputed on host), so the head+feedback hop disappears from the serial
chain; the visible outputs are recovered by a streamed head GEMM over the
stored h3 states. The decoder runs as two independent half-batch chains
whose hops interleave, halving the act size and chain window per hop.
"""
import numpy as np
from contextlib import ExitStack

import concourse.bacc as bacc
import concourse.tile as tile
from concourse import mybir
from concourse.bass_utils import run_bass_kernel_spmd

B, T, IN, H, TGT = 128, 128, 3, 512, 32
NC = 8
BC = B // NC          # 16 batch rows per core
CH = H // 128         # 4 chunks of the hidden dim
F16 = mybir.dt.float16
F32 = mybir.dt.float32
Tanh = mybir.ActivationFunctionType.Tanh

# smalls tile column offsets (fp16 [128, C_SMALL])
B1D0, B1D1 = 0, 512            # enc l1 bias rows (row 0)
DB = 1024                      # dec l1..3 bias rows (row 0), 512 each
C0 = 2560                      # dec l0 const row (row 0)
CS = 3072                      # xin const row [1,2] (row 0)
B2C = 3074                     # dec l0 xin coeffs [2,512] (rows 0-1)
S2C = 3586                     # xin xin-coeffs [2,2] (rows 0-1)
DIN0 = 3588                    # dec l0 t=0 stationary [4,512] (rows 0-3)
XQ = 4100                      # per-core x-init [4,16] rows (x0,x1,1,x2)
LINC = 4116                    # lin head chunks [128,4]
WX2 = 4120                     # xin h3-coeff chunks [128,8]
ONES = 4128                    # all-ones [128,16]
IDC = 4144                     # identity [128,128]
LB = 4272                      # lin_b scalar (row 0)
C_SMALL = 4274

_prog_cache = {}


def _build_program():
    if "nc" in _prog_cache:
        return _prog_cache["nc"]
    nc = bacc.Bacc("TRN2")
    dp = nc.declare_dram_parameter

    # encoder Whh weights are double-fp16 (hi+lo) pairs: fp16 rounding of the
    # recurrent weights is a systematic perturbation that dominates the final
    # error (1.3e-2 alone); the lo-correction matmuls bring it back to ~5e-3.
    ident_e = dp("ident", [128, 128], F16, isOutput=False)
    whh0_e = dp("whh0", [128, 4 * 2048], F16, isOutput=False)
    xs0_e = dp("xs0", [2, 128, T * 4 * BC], F16, isOutput=False)
    wenc1_e = dp("wenc1", [128, 2 * 2048 + 2 * 4096], F16, isOutput=False)
    wdec_e = dp("wdec", [128, 8 * 2048], F16, isOutput=False)
    smalls_e = dp("smalls", [128, C_SMALL], F16, isOutput=False)
    out_e = dp("out", [1, TGT * BC], F32, isOutput=True)

    SW = T * 4 * BC  # 8192 cols per direction

    with tile.TileContext(nc) as tc, ExitStack() as ctx:
        wpool = ctx.enter_context(tc.tile_pool(name="w", bufs=1))
        hpool = ctx.enter_context(tc.tile_pool(name="h", bufs=1))
        pspool = ctx.enter_context(tc.tile_pool(name="ps", bufs=1, space="PSUM"))

        whh0s = wpool.tile([128, 4 * 2048], F16)   # enc l0 Whh.T (d, hi|lo)
        xs0 = wpool.tile([128, 2 * SW], F16)       # l0 x-proj(+bias), feature-major
        wenc1 = wpool.tile([128, 2 * 2048 + 2 * 4096], F16)  # whh1 (2) | wih1 (2)
        wdec = wpool.tile([128, 8 * 2048], F16)    # dwhh(4) | dwihr(3) | A
        smalls = wpool.tile([128, C_SMALL], F16)
        hbuf0 = {d: wpool.tile([128, SW], F16, name=f"hbuf0_{d}") for d in range(2)}
        hbuf3 = wpool.tile([128, TGT * 4 * BC], F16)   # dec l3 states per t

        # All DMAs go on ONE queue in exact need-order: the sim serializes
        # transfers on a single DMA resource by arrival, so a second queue
        # just lets a bulk weight cut ahead of the small tiles that gate the
        # first activation (E0 is Act-bound, so first-act time is wall time).
        identt = wpool.tile([128, 128], F16, name="identt")
        nc.sync.dma_start(identt[:], ident_e[:])

        def xs_chunk(i, bounds=[0, 2, 8, 16, 32, 64, 96, 128]):
            a, b = bounds[i] * 64, bounds[i + 1] * 64
            for d in range(2):
                nc.sync.dma_start(xs0[:, d * SW + a:d * SW + b],
                                  xs0_e[d, :, a:b])

        xs_chunk(0)  # 2 steps: first activation gates only on this + ident
        xs_chunk(1)
        nc.sync.dma_start(whh0s[:, 0:4096], whh0_e[:, 0:4096])
        nc.sync.dma_start(whh0s[:, 4096:8192], whh0_e[:, 4096:8192])
        xs_chunk(2)
        xs_chunk(3)
        nc.sync.dma_start(wenc1[:], wenc1_e[:])
        xs_chunk(4)
        nc.sync.dma_start(wdec[:], wdec_e[:])
        xs_chunk(5)
        xs_chunk(6)
        nc.sync.dma_start(smalls[:], smalls_e[:])

        ident = identt[:]
        ones1 = smalls[0:1, ONES:ONES + 16]

        def mm(ps_ap, lhsT_ap, rhs_ap, start, stop):
            nc.tensor.matmul(ps_ap, lhsT_ap, rhs_ap, start=start, stop=stop)

        # ---- encoder layer 0: four interleaved chains (2 dirs x 2 batch
        # halves of 8 rows). hbuf0 column order is t*64 + g*32 + kc*8 + j so
        # every chain's act output and matmul operand is a contiguous slice.
        # Half-batch halves the act size and the recurrent matmul window.
        e0ps = {}

        def e0_ready(t, d, g):
            ps = pspool.tile([128, 512], F32, tag=f"ps{d}{g}", name=f"psE{d}{g}", bufs=2)
            e0ps[(t, d, g)] = ps
            xsl = xs0[:, d * SW + t * 64 + 32 * g:d * SW + t * 64 + 32 * (g + 1)]
            mm(ps[:, 0:32], ident, xsl, True, t == 0)

        for d in range(2):
            for g in range(2):
                e0_ready(0, d, g)
        for t in range(T):
            for d in range(2):
                for g in range(2):
                    ps = e0ps.pop((t, d, g))
                    if t > 0:
                        hb = (t - 1) * 64 + 32 * g
                        for kc in range(CH):
                            for nb in range(CH):
                                for part in range(2):  # hi then lo correction
                                    o = d * 4096 + part * 2048 + kc * 512 + nb * 128
                                    mm(ps[:, 8 * nb:8 * (nb + 1)],
                                       whh0s[:, o:o + 128],
                                       hbuf0[d][:, hb + 8 * kc:hb + 8 * (kc + 1)],
                                       False, kc == CH - 1 and nb == CH - 1 and part == 1)
                    nc.scalar.activation(hbuf0[d][:, t * 64 + 32 * g:t * 64 + 32 * (g + 1)],
                                         ps[:, 0:32], Tanh)
                    if t + 1 < T:
                        e0_ready(t + 1, d, g)

        # ---- encoder layer 1: fused input projection from hbuf0 ----
        WIH1 = 2 * 2048  # offset of wih1 region inside wenc1
        e1h = {}
        e1ps = {}

        def e1_ready(t, d):
            """Bias + fused input-projection matmuls for step t of chain d --
            no dependence on the chain, issued one step ahead so they never
            sit in the PE wait queue behind the recurrent matmuls."""
            f_slot = t if d == 0 else T - 1 - t
            b_slot = T - 1 - t if d == 0 else t
            ps = pspool.tile([128, 512], F32, tag=f"ps{d}0", name=f"psF{d}", bufs=2)
            e1ps[(t, d)] = ps
            for nb in range(CH):
                mm(ps[:, 16 * nb:16 * (nb + 1)],
                   smalls[0:1, d * 512 + nb * 128:d * 512 + (nb + 1) * 128],
                   ones1, nb == 0, False)
            for k8 in range(2 * CH):
                src = hbuf0[0] if k8 < CH else hbuf0[1]
                slot = f_slot if k8 < CH else b_slot
                for g in range(2):  # hbuf0 is half-major: one mm per half
                    rhs = src[:, slot * 64 + 32 * g + 8 * (k8 % CH):slot * 64 + 32 * g + 8 * (k8 % CH + 1)]
                    for nb in range(CH):
                        mm(ps[:, 16 * nb + 8 * g:16 * nb + 8 * (g + 1)],
                           wenc1[:, WIH1 + d * 4096 + k8 * 512 + nb * 128:WIH1 + d * 4096 + k8 * 512 + (nb + 1) * 128],
                           rhs, False,
                           t == 0 and k8 == 2 * CH - 1 and g == 1 and nb == CH - 1)

        for d in range(2):
            e1_ready(0, d)
        for t in range(T):
            for d in range(2):
                # both chains' recurrent matmuls issue before either chain's
                # next ready burst: the engine drains in issue order, so a
                # burst issued between them would stall the second chain's
                # whh behind ~68 ready pairs every step.
                ps = e1ps[(t, d)]
                if t > 0:
                    hprev = e1h[d][:, 0:64]
                    for kc in range(CH):
                        for nb in range(CH):
                            o = d * 2048 + kc * 512 + nb * 128
                            mm(ps[:, 16 * nb:16 * (nb + 1)],
                               wenc1[:, o:o + 128],
                               hprev[:, 16 * kc:16 * (kc + 1)],
                               False, kc == CH - 1 and nb == CH - 1)
                hnew = hpool.tile([128, 64], F16, tag=f"e1_{d}", name=f"e1_{d}", bufs=2)
                nc.scalar.activation(hnew[:], ps[:, 0:64], Tanh)
                e1h[d] = hnew
            for d in range(2):
                del e1ps[(t, d)]
                if t + 1 < T:
                    e1_ready(t + 1, d)
            if t == 64:
                # decoder layers 0/1 of step 0 depend only on E0 finals and
                # the x-init: emit them mid-E1 so they execute in E1's engine
                # slack on the two PSUM banks E1 doesn't use (ps01/ps11),
                # removing two serial hops from the decoder phase.
                pre_h = {}
                o8 = smalls[0:1, ONES:ONES + 8]
                for g in range(2):
                    psp = pspool.tile([128, 512], F32, tag="ps01", name=f"pre0{g}", bufs=2)
                    for nb in range(CH):
                        mm(psp[:, 8 * nb:8 * (nb + 1)],
                           smalls[0:4, DIN0 + nb * 128:DIN0 + (nb + 1) * 128],
                           smalls[0:4, XQ + 8 * g:XQ + 8 * g + 8], nb == 0, False)
                    for kc in range(CH):
                        for nb in range(CH):
                            mm(psp[:, 8 * nb:8 * (nb + 1)],
                               wdec[:, kc * 512 + nb * 128:kc * 512 + (nb + 1) * 128],
                               hbuf0[0][:, (T - 1) * 64 + 32 * g + 8 * kc:(T - 1) * 64 + 32 * g + 8 * kc + 8],
                               False, kc == CH - 1 and nb == CH - 1)
                    h0p = hpool.tile([128, 32], F16, tag=f"hd0_{g}", name=f"hd0p{g}", bufs=2)
                    nc.scalar.activation(h0p[:], psp[:, 0:32], Tanh)
                    pre_h[(0, g)] = h0p
                for g in range(2):
                    psp = pspool.tile([128, 512], F32, tag="ps11", name=f"pre1{g}", bufs=2)
                    for nb in range(CH):
                        mm(psp[:, 8 * nb:8 * (nb + 1)],
                           smalls[0:1, 1024 + nb * 128:1024 + (nb + 1) * 128],
                           o8, nb == 0, False)
                    for kc in range(CH):
                        for nb in range(CH):
                            mm(psp[:, 8 * nb:8 * (nb + 1)],
                               wdec[:, 1 * 2048 + kc * 512 + nb * 128:1 * 2048 + kc * 512 + (nb + 1) * 128],
                               hbuf0[1][:, (T - 1) * 64 + 32 * g + 8 * kc:(T - 1) * 64 + 32 * g + 8 * kc + 8],
                               False, False)
                    for kc in range(CH):
                        for nb in range(CH):
                            mm(psp[:, 8 * nb:8 * (nb + 1)],
                               wdec[:, 4 * 2048 + kc * 512 + nb * 128:4 * 2048 + kc * 512 + (nb + 1) * 128],
                               pre_h[(0, g)][:, 8 * kc:8 * kc + 8],
                               False, kc == CH - 1 and nb == CH - 1)
                    h1p = hpool.tile([128, 32], F16, tag=f"hd1_{g}", name=f"hd1p{g}", bufs=2)
                    nc.scalar.activation(h1p[:], psp[:, 0:32], Tanh)
                    pre_h[(1, g)] = h1p

        # ---- decoder: 4-layer stack, 32 autoregressive steps ----
        # Split into two independent half-batch chains (8 rows each): the
        # halves' hops interleave like the encoder directions, halving both
        # the activation size and the serial matmul window per hop.
        DWIHR = 4 * 2048
        AOFF = 7 * 2048
        ph = None
        outt = hpool.tile([1, TGT * BC], F32, tag="out", name="out")

        def _e0_src(tile_, base):  # hbuf0 half-major layout
            return lambda g, kc: tile_[:, base + 32 * g + 8 * kc:base + 32 * g + 8 * kc + 8]

        def _enc_src(tile_, base):  # e1h batch-major layout
            return lambda g, kc: tile_[:, base + 16 * kc + 8 * g:base + 16 * kc + 8 * g + 8]

        hsrc = {0: lambda g, kc: pre_h[(0, g)][:, 8 * kc:8 * kc + 8],
                1: lambda g, kc: pre_h[(1, g)][:, 8 * kc:8 * kc + 8],
                2: _enc_src(e1h[0], 0), 3: _enc_src(e1h[1], 0)}

        def h3c(t, g, kc):
            return hbuf3[:, t * 64 + 32 * g + 8 * kc:t * 64 + 32 * g + 8 * kc + 8]

        xqf = lambda g: smalls[0:2, XQ + 8 * g:XQ + 8 * g + 8]
        ones8 = smalls[0:1, ONES:ONES + 8]
        for t in range(TGT):
            lays = range(2, 4) if t == 0 else range(4)  # l0/l1 at t=0 pre-ran in E1
            pss = {}
            for l in lays:
                for g in range(2):
                    pss[(l, g)] = pspool.tile(
                        [128, 512], F32, tag=f"ps0{g}",
                        name=f"psD{l}_{g}", bufs=2)
            # ready work: layer 0 xin terms + all layers' bias/Whh
            if t > 0:
                for g in range(2):
                    ps = pss[(0, g)]
                    for nb in range(CH):
                        mm(ps[:, 8 * nb:8 * (nb + 1)],
                           smalls[0:2, B2C + nb * 128:B2C + (nb + 1) * 128],
                           xqf(g), nb == 0, False)
                        mm(ps[:, 8 * nb:8 * (nb + 1)],
                           smalls[0:1, C0 + nb * 128:C0 + (nb + 1) * 128],
                           ones8, False, False)
                    for kc in range(CH):
                        for nb in range(CH):
                            mm(ps[:, 8 * nb:8 * (nb + 1)],
                               wdec[:, kc * 512 + nb * 128:kc * 512 + (nb + 1) * 128],
                               hsrc[0](g, kc), False, False)
            # layer 0 chain input (A @ h3_{t-1}) issued BEFORE the l1-3 ready
            # bursts: when act3_{t-1} fires, A is at the wait-queue head
            # instead of behind ~150 ready matmul pairs.
            if t > 0:
                for g in range(2):
                    ps = pss[(0, g)]
                    for kc in range(CH):
                        for nb in range(CH):
                            mm(ps[:, 8 * nb:8 * (nb + 1)],
                               wdec[:, AOFF + kc * 512 + nb * 128:AOFF + kc * 512 + (nb + 1) * 128],
                               h3c(t - 1, g, kc),
                               False, kc == CH - 1 and nb == CH - 1)
            for l in range(2 if t == 0 else 1, 4):
                for g in range(2):
                    ps = pss[(l, g)]
                    for nb in range(CH):
                        mm(ps[:, 8 * nb:8 * (nb + 1)],
                           smalls[0:1, DB + (l - 1) * 512 + nb * 128:DB + (l - 1) * 512 + (nb + 1) * 128],
                           ones8, nb == 0, False)
                    for kc in range(CH):
                        for nb in range(CH):
                            mm(ps[:, 8 * nb:8 * (nb + 1)],
                               wdec[:, l * 2048 + kc * 512 + nb * 128:l * 2048 + kc * 512 + (nb + 1) * 128],
                               hsrc[l](g, kc), False, False)
            if t > 0:
                ht0 = {}
                for g in range(2):
                    h0 = hpool.tile([128, 32], F16, tag=f"hd0_{g}", name=f"hd0_{g}", bufs=2)
                    nc.scalar.activation(h0[:], pss[(0, g)][:, 0:32], Tanh)
                    ht0[g] = h0
                hsrc[0] = lambda g, kc, _h=ht0: _h[g][:, 8 * kc:8 * kc + 8]
            # px/head fill the act0 wait (same trigger: act3_{t-1})
            if 1 <= t < TGT - 1:
                px = pspool.tile([128, 512], F32, tag="ps10", name="psX", bufs=2)
                for g in range(2):
                    for kc in range(CH):
                        mm(px[0:2, 8 * g:8 * g + 8],
                           smalls[:, WX2 + 2 * kc:WX2 + 2 * (kc + 1)],
                           h3c(t - 1, g, kc), g == 0 and kc == 0, False)
                    mm(px[0:2, 8 * g:8 * g + 8], smalls[0:2, S2C:S2C + 2],
                       xqf(g), False, False)
                    mm(px[0:2, 8 * g:8 * g + 8], smalls[0:1, CS:CS + 2],
                       ones8, False, g == 1)
                xnew = hpool.tile([2, 16], F16, tag="xin", name="xin", bufs=2)
                nc.vector.tensor_copy(xnew[:], px[0:2, 0:16])
            if t >= 1:
                # head rows stream in 8-step chunks, each its own group on an
                # alternating ph slot, evacuated + DMA'd as soon as complete
                # so only the final chunk sits after the last decoder step.
                if (t - 1) % 8 == 0:
                    ph = pspool.tile([128, 512], F32, tag="ps11", name="psH", bufs=2)
                for g in range(2):
                    for kc in range(CH):
                        mm(ph[0:1, 16 * (t - 1) + 8 * g:16 * (t - 1) + 8 * g + 8],
                           smalls[:, LINC + kc:LINC + kc + 1],
                           h3c(t - 1, g, kc), (t - 1) % 8 == 0 and g == 0 and kc == 0,
                           (t - 1) % 8 == 7 and g == 1 and kc == CH - 1)
                if (t - 1) % 8 == 7:
                    c = (t - 1) // 8
                    nc.scalar.activation(outt[0:1, 128 * c:128 * (c + 1)],
                                         ph[0:1, 128 * c:128 * (c + 1)],
                                         mybir.ActivationFunctionType.Identity,
                                         bias=smalls[0:1, LB:LB + 1])
                    nc.sync.dma_start(out_e[:, 128 * c:128 * (c + 1)],
                                      outt[0:1, 128 * c:128 * (c + 1)])
            # same-step Wih chains
            for l in range(2 if t == 0 else 1, 4):
                htl = {}
                for g in range(2):
                    ps = pss[(l, g)]
                    for kc in range(CH):
                        for nb in range(CH):
                            mm(ps[:, 8 * nb:8 * (nb + 1)],
                               wdec[:, DWIHR + (l - 1) * 2048 + kc * 512 + nb * 128:DWIHR + (l - 1) * 2048 + kc * 512 + (nb + 1) * 128],
                               hsrc[l - 1](g, kc),
                               False, kc == CH - 1 and nb == CH - 1)
                for g in range(2):
                    if l == 3:
                        nc.scalar.activation(hbuf3[:, t * 64 + 32 * g:t * 64 + 32 * (g + 1)],
                                             pss[(l, g)][:, 0:32], Tanh)
                    else:
                        hl = hpool.tile([128, 32], F16, tag=f"hd{l}_{g}", name=f"hd{l}_{g}", bufs=2)
                        nc.scalar.activation(hl[:], pss[(l, g)][:, 0:32], Tanh)
                        htl[g] = hl
                if l == 3:
                    hsrc[3] = lambda g, kc, _t=t: h3c(_t, g, kc)
                else:
                    hsrc[l] = lambda g, kc, _h=dict(htl): _h[g][:, 8 * kc:8 * kc + 8]
            if 1 <= t < TGT - 1:
                xqf = lambda g, _x=xnew: _x[0:2, 8 * g:8 * g + 8]
        for g in range(2):
            for kc in range(CH):  # head row for the final step (chunk 3 stop)
                mm(ph[0:1, 16 * (TGT - 1) + 8 * g:16 * (TGT - 1) + 8 * g + 8],
                   smalls[:, LINC + kc:LINC + kc + 1],
                   h3c(TGT - 1, g, kc), False, g == 1 and kc == CH - 1)
        nc.scalar.activation(outt[0:1, 384:512], ph[0:1, 384:512],
                             mybir.ActivationFunctionType.Identity,
                             bias=smalls[0:1, LB:LB + 1])
        nc.sync.dma_start(out_e[:, 384:512], outt[0:1, 384:512])

    nc.compile()
    _prog_cache["nc"] = nc
    return nc


def _statT(W):
    """W (N,K), h_new = W @ h -> stationary tile [128, (K//128)*N]:
    chunk kc at cols [kc*N:(kc+1)*N] holds W.T[128*kc:128*(kc+1), :]."""
    W = np.asarray(W, np.float32)
    N, K = W.shape
    WT = np.ascontiguousarray(W.T)
    return WT.reshape(K // 128, 128, N).transpose(1, 0, 2).reshape(128, (K // 128) * N)


def kernel(x, y, enc_Wih0, enc_Whh0, enc_Wih1, enc_Whh1, enc_bih, enc_bhh,
           dec_Wih0, dec_Wihr, dec_Whh, dec_bih, dec_bhh, lin_W, lin_b,
           target_len, teacher_forcing_ratio):
    f, h16 = np.float32, np.float16
    x = np.asarray(x, f)
    enc_Wih0, enc_Whh0 = np.asarray(enc_Wih0, f), np.asarray(enc_Whh0, f)
    enc_Wih1, enc_Whh1 = np.asarray(enc_Wih1, f), np.asarray(enc_Whh1, f)
    enc_bih, enc_bhh = np.asarray(enc_bih, f), np.asarray(enc_bhh, f)
    dec_Wih0, dec_Wihr = np.asarray(dec_Wih0, f), np.asarray(dec_Wihr, f)
    dec_Whh = np.asarray(dec_Whh, f)
    dec_bih, dec_bhh = np.asarray(dec_bih, f), np.asarray(dec_bhh, f)
    lin_W = np.asarray(lin_W, f)
    lb = float(np.asarray(lin_b, f).reshape(()))

    def _hilo(W):
        hi = W.astype(h16).astype(f)
        return [_statT(hi), _statT(W - hi)]

    whh0 = np.concatenate(_hilo(enc_Whh0[0]) + _hilo(enc_Whh0[1]), 1).astype(h16)
    wenc1 = np.concatenate([_statT(enc_Whh1[d]) for d in range(2)]
                           + [_statT(enc_Wih1[d]) for d in range(2)], 1).astype(h16)

    W0, linv = dec_Wih0, lin_W[0]  # (512,3), (512,)
    Nv = np.array([1.0, -1.0, 1.0], f)
    A = np.outer(W0 @ Nv, linv)                      # (512,512)
    b0tot = dec_bih[0] + dec_bhh[0]
    c0 = (W0 @ Nv) * lb + b0tot                      # (512,)
    B2 = np.stack([W0[:, 1] - W0[:, 2], W0[:, 2]])   # (2,512): x0,x1 coeffs
    wdec = np.concatenate([_statT(dec_Whh[l]) for l in range(4)]
                          + [_statT(dec_Wihr[l]) for l in range(3)]
                          + [_statT(A)], 1).astype(h16)

    smalls = np.zeros((128, C_SMALL), f)
    for d in range(2):
        smalls[0, d * 512:(d + 1) * 512] = enc_bih[1, d] + enc_bhh[1, d]
    for l in range(1, 4):
        smalls[0, DB + (l - 1) * 512:DB + l * 512] = dec_bih[l] + dec_bhh[l]
    smalls[0, C0:C0 + 512] = c0
    smalls[0, CS:CS + 2] = [lb, -lb]
    smalls[0:2, B2C:B2C + 512] = B2
    smalls[0:2, S2C:S2C + 2] = np.array([[0, 1], [0, 0]], f)
    din0q = np.zeros((4, 512), f)   # rows match xq rows (x0, x1, 1, x2)
    din0q[0], din0q[1], din0q[3] = W0[:, 0], W0[:, 1], W0[:, 2]
    din0q[2] = b0tot
    smalls[0:4, DIN0:DIN0 + 512] = din0q
    smalls[:, LINC:LINC + 4] = linv.reshape(4, 128).T
    wx2 = np.stack([linv, -linv])                    # (2,512)
    smalls[:, WX2:WX2 + 8] = wx2.T.reshape(4, 128, 2).transpose(1, 0, 2).reshape(128, 8)
    smalls[:, ONES:ONES + 16] = 1.0
    smalls[:, IDC:IDC + 128] = np.eye(128, dtype=f)
    smalls[0, LB] = lb

    nc = _build_program()

    in_maps = []
    for c in range(NC):
        xc = x[c * BC:(c + 1) * BC]  # (16, T, 3)
        xs0 = np.empty((2, 128, T * 4 * BC), h16)
        for d in range(2):
            W = enc_Wih0[d]
            b = enc_bih[0, d] + enc_bhh[0, d]
            proj = np.einsum('bti,hi->tbh', xc, W) + b   # (T, 16, 512)
            if d == 1:
                proj = proj[::-1]
            # half-major columns: t*64 + g*32 + kc*8 + j  (batch b = 8g+j)
            xs0[d] = proj.reshape(T, 2, BC // 2, 4, 128).transpose(4, 0, 1, 3, 2).reshape(128, T * 4 * BC)
        sm = smalls.copy()
        sm[0:2, XQ:XQ + 16] = xc[:, -1, 0:2].T
        sm[2, XQ:XQ + 16] = 1.0
        sm[3, XQ:XQ + 16] = xc[:, -1, 2]
        in_maps.append({
            "ident": np.eye(128, dtype=h16),
            "whh0": whh0, "xs0": xs0, "wenc1": wenc1, "wdec": wdec,
            "smalls": sm.astype(h16),
        })
    res = run_bass_kernel_spmd(nc, in_maps, list(range(NC)))
    out = np.stack([res.results[c]["out"].reshape(TGT, BC).T for c in range(NC)])
    return out.reshape(B, TGT, 1).astype(np.float32)


# revision 61
# speedup vs baseline: 1.0036x; 1.0036x over previous
"""BiRNN encoder-decoder Trainium2 kernel, feature-major layout.

Data-parallel over batch (8 cores x 16 rows). All state is kept
feature-major: h lives in SBUF as [128 (H-chunk), 16 (batch)] fp16 columns,
weights are the PE stationary operand ([k-chunk, n-chunk] tiles of W.T) and
the state is the moving operand, so each recurrent matmul's cost scales with
the 16-wide batch (free size) instead of the 512-wide hidden dim. No
transposes anywhere: the PSUM output [128n, 16b] of one step is exactly the
moving layout the next step needs; tanh evacuates PSUM->SBUF directly.

Decoder feedback is algebraically folded into the layer-0 matmul: with
o0 = lin.h3 + lb and nxt = [o0, x0-o0, x1-x0+o0], layer-0's next-step input
projection W0.nxt becomes A.h3 + B2.[x0;x1] + c0 with A = W0.N.lin (rank-1,
precomputed on host), so the head+feedback hop disappears from the serial
chain; the visible outputs are recovered by a streamed head GEMM over the
stored h3 states. The decoder runs as two independent half-batch chains
whose hops interleave, halving the act size and chain window per hop.
"""
import numpy as np
from contextlib import ExitStack

import concourse.bacc as bacc
import concourse.tile as tile
from concourse import mybir
from concourse.bass_utils import run_bass_kernel_spmd

B, T, IN, H, TGT = 128, 128, 3, 512, 32
NC = 8
BC = B // NC          # 16 batch rows per core
CH = H // 128         # 4 chunks of the hidden dim
F16 = mybir.dt.float16
F32 = mybir.dt.float32
Tanh = mybir.ActivationFunctionType.Tanh

# smalls tile column offsets (fp16 [128, C_SMALL])
B1D0, B1D1 = 0, 512            # enc l1 bias rows (row 0)
DB = 1024                      # dec l1..3 bias rows (row 0), 512 each
C0 = 2560                      # dec l0 const row (row 0)
CS = 3072                      # xin const row [1,2] (row 0)
B2C = 3074                     # dec l0 xin coeffs [2,512] (rows 0-1)
S2C = 3586                     # xin xin-coeffs [2,2] (rows 0-1)
DIN0 = 3588                    # dec l0 t=0 stationary [4,512] (rows 0-3)
XQ = 4100                      # per-core x-init [4,16] rows (x0,x1,1,x2)
LINC = 4116                    # lin head chunks [128,4]
WX2 = 4120                     # xin h3-coeff chunks [128,8]
ONES = 4128                    # all-ones [128,16]
IDC = 4144                     # identity [128,128]
LB = 4272                      # lin_b scalar (row 0)
C_SMALL = 4274

_prog_cache = {}


def _build_program():
    if "nc" in _prog_cache:
        return _prog_cache["nc"]
    nc = bacc.Bacc("TRN2")
    dp = nc.declare_dram_parameter

    # encoder Whh weights are double-fp16 (hi+lo) pairs: fp16 rounding of the
    # recurrent weights is a systematic perturbation that dominates the final
    # error (1.3e-2 alone); the lo-correction matmuls bring it back to ~5e-3.
    ident_e = dp("ident", [128, 128], F16, isOutput=False)
    whh0_e = dp("whh0", [128, 4 * 2048], F16, isOutput=False)
    xs0_e = dp("xs0", [2, 128, T * 4 * BC], F16, isOutput=False)
    wenc1_e = dp("wenc1", [128, 2 * 2048 + 2 * 4096], F16, isOutput=False)
    wdec_e = dp("wdec", [128, 8 * 2048], F16, isOutput=False)
    smalls_e = dp("smalls", [128, C_SMALL], F16, isOutput=False)
    out_e = dp("out", [1, TGT * BC], F32, isOutput=True)

    SW = T * 4 * BC  # 8192 cols per direction

    with tile.TileContext(nc) as tc, ExitStack() as ctx:
        wpool = ctx.enter_context(tc.tile_pool(name="w", bufs=1))
        hpool = ctx.enter_context(tc.tile_pool(name="h", bufs=1))
        pspool = ctx.enter_context(tc.tile_pool(name="ps", bufs=1, space="PSUM"))

        whh0s = wpool.tile([128, 4 * 2048], F16)   # enc l0 Whh.T (d, hi|lo)
        xs0 = wpool.tile([128, 2 * SW], F16)       # l0 x-proj(+bias), feature-major
        wenc1 = wpool.tile([128, 2 * 2048 + 2 * 4096], F16)  # whh1 (2) | wih1 (2)
        wdec = wpool.tile([128, 8 * 2048], F16)    # dwhh(4) | dwihr(3) | A
        smalls = wpool.tile([128, C_SMALL], F16)
        hbuf0 = {d: wpool.tile([128, SW], F16, name=f"hbuf0_{d}") for d in range(2)}
        hbuf3 = wpool.tile([128, TGT * 4 * BC], F16)   # dec l3 states per t

        # All DMAs go on ONE queue in exact need-order: the sim serializes
        # transfers on a single DMA resource by arrival, so a second queue
        # just lets a bulk weight cut ahead of the small tiles that gate the
        # first activation (E0 is Act-bound, so first-act time is wall time).
        identt = wpool.tile([128, 128], F16, name="identt")
        nc.sync.dma_start(identt[:], ident_e[:])

        def xs_chunk(i, bounds=[0, 8, 16, 32, 64, 96, 128]):
            a, b = bounds[i] * 64, bounds[i + 1] * 64
            for d in range(2):
                nc.sync.dma_start(xs0[:, d * SW + a:d * SW + b],
                                  xs0_e[d, :, a:b])

        xs_chunk(0)
        nc.sync.dma_start(whh0s[:, 0:4096], whh0_e[:, 0:4096])
        nc.sync.dma_start(whh0s[:, 4096:8192], whh0_e[:, 4096:8192])
        xs_chunk(1)
        xs_chunk(2)
        nc.sync.dma_start(wenc1[:], wenc1_e[:])
        xs_chunk(3)
        nc.sync.dma_start(wdec[:], wdec_e[:])
        xs_chunk(4)
        xs_chunk(5)
        nc.sync.dma_start(smalls[:], smalls_e[:])

        ident = identt[:]
        ones1 = smalls[0:1, ONES:ONES + 16]

        def mm(ps_ap, lhsT_ap, rhs_ap, start, stop):
            nc.tensor.matmul(ps_ap, lhsT_ap, rhs_ap, start=start, stop=stop)

        # ---- encoder layer 0: four interleaved chains (2 dirs x 2 batch
        # halves of 8 rows). hbuf0 column order is t*64 + g*32 + kc*8 + j so
        # every chain's act output and matmul operand is a contiguous slice.
        # Half-batch halves the act size and the recurrent matmul window.
        e0ps = {}

        def e0_ready(t, d, g):
            ps = pspool.tile([128, 512], F32, tag=f"ps{d}{g}", name=f"psE{d}{g}", bufs=2)
            e0ps[(t, d, g)] = ps
            xsl = xs0[:, d * SW + t * 64 + 32 * g:d * SW + t * 64 + 32 * (g + 1)]
            mm(ps[:, 0:32], ident, xsl, True, t == 0)

        for d in range(2):
            for g in range(2):
                e0_ready(0, d, g)
        for t in range(T):
            for d in range(2):
                for g in range(2):
                    ps = e0ps.pop((t, d, g))
                    if t > 0:
                        hb = (t - 1) * 64 + 32 * g
                        for kc in range(CH):
                            for nb in range(CH):
                                for part in range(2):  # hi then lo correction
                                    o = d * 4096 + part * 2048 + kc * 512 + nb * 128
                                    mm(ps[:, 8 * nb:8 * (nb + 1)],
                                       whh0s[:, o:o + 128],
                                       hbuf0[d][:, hb + 8 * kc:hb + 8 * (kc + 1)],
                                       False, kc == CH - 1 and nb == CH - 1 and part == 1)
                    nc.scalar.activation(hbuf0[d][:, t * 64 + 32 * g:t * 64 + 32 * (g + 1)],
                                         ps[:, 0:32], Tanh)
                    if t + 1 < T:
                        e0_ready(t + 1, d, g)

        # ---- encoder layer 1: fused input projection from hbuf0 ----
        WIH1 = 2 * 2048  # offset of wih1 region inside wenc1
        e1h = {}
        e1ps = {}

        def e1_ready(t, d):
            """Bias + fused input-projection matmuls for step t of chain d --
            no dependence on the chain, issued one step ahead so they never
            sit in the PE wait queue behind the recurrent matmuls."""
            f_slot = t if d == 0 else T - 1 - t
            b_slot = T - 1 - t if d == 0 else t
            ps = pspool.tile([128, 512], F32, tag=f"ps{d}0", name=f"psF{d}", bufs=2)
            e1ps[(t, d)] = ps
            for nb in range(CH):
                mm(ps[:, 16 * nb:16 * (nb + 1)],
                   smalls[0:1, d * 512 + nb * 128:d * 512 + (nb + 1) * 128],
                   ones1, nb == 0, False)
            for k8 in range(2 * CH):
                src = hbuf0[0] if k8 < CH else hbuf0[1]
                slot = f_slot if k8 < CH else b_slot
                for g in range(2):  # hbuf0 is half-major: one mm per half
                    rhs = src[:, slot * 64 + 32 * g + 8 * (k8 % CH):slot * 64 + 32 * g + 8 * (k8 % CH + 1)]
                    for nb in range(CH):
                        mm(ps[:, 16 * nb + 8 * g:16 * nb + 8 * (g + 1)],
                           wenc1[:, WIH1 + d * 4096 + k8 * 512 + nb * 128:WIH1 + d * 4096 + k8 * 512 + (nb + 1) * 128],
                           rhs, False,
                           t == 0 and k8 == 2 * CH - 1 and g == 1 and nb == CH - 1)

        for d in range(2):
            e1_ready(0, d)
        for t in range(T):
            for d in range(2):
                # both chains' recurrent matmuls issue before either chain's
                # next ready burst: the engine drains in issue order, so a
                # burst issued between them would stall the second chain's
                # whh behind ~68 ready pairs every step.
                ps = e1ps[(t, d)]
                if t > 0:
                    hprev = e1h[d][:, 0:64]
                    for kc in range(CH):
                        for nb in range(CH):
                            o = d * 2048 + kc * 512 + nb * 128
                            mm(ps[:, 16 * nb:16 * (nb + 1)],
                               wenc1[:, o:o + 128],
                               hprev[:, 16 * kc:16 * (kc + 1)],
                               False, kc == CH - 1 and nb == CH - 1)
                hnew = hpool.tile([128, 64], F16, tag=f"e1_{d}", name=f"e1_{d}", bufs=2)
                nc.scalar.activation(hnew[:], ps[:, 0:64], Tanh)
                e1h[d] = hnew
            for d in range(2):
                del e1ps[(t, d)]
                if t + 1 < T:
                    e1_ready(t + 1, d)
            if t == 64:
                # decoder layers 0/1 of step 0 depend only on E0 finals and
                # the x-init: emit them mid-E1 so they execute in E1's engine
                # slack on the two PSUM banks E1 doesn't use (ps01/ps11),
                # removing two serial hops from the decoder phase.
                pre_h = {}
                o8 = smalls[0:1, ONES:ONES + 8]
                for g in range(2):
                    psp = pspool.tile([128, 512], F32, tag="ps01", name=f"pre0{g}", bufs=2)
                    for nb in range(CH):
                        mm(psp[:, 8 * nb:8 * (nb + 1)],
                           smalls[0:4, DIN0 + nb * 128:DIN0 + (nb + 1) * 128],
                           smalls[0:4, XQ + 8 * g:XQ + 8 * g + 8], nb == 0, False)
                    for kc in range(CH):
                        for nb in range(CH):
                            mm(psp[:, 8 * nb:8 * (nb + 1)],
                               wdec[:, kc * 512 + nb * 128:kc * 512 + (nb + 1) * 128],
                               hbuf0[0][:, (T - 1) * 64 + 32 * g + 8 * kc:(T - 1) * 64 + 32 * g + 8 * kc + 8],
                               False, kc == CH - 1 and nb == CH - 1)
                    h0p = hpool.tile([128, 32], F16, tag=f"hd0_{g}", name=f"hd0p{g}", bufs=2)
                    nc.scalar.activation(h0p[:], psp[:, 0:32], Tanh)
                    pre_h[(0, g)] = h0p
                for g in range(2):
                    psp = pspool.tile([128, 512], F32, tag="ps11", name=f"pre1{g}", bufs=2)
                    for nb in range(CH):
                        mm(psp[:, 8 * nb:8 * (nb + 1)],
                           smalls[0:1, 1024 + nb * 128:1024 + (nb + 1) * 128],
                           o8, nb == 0, False)
                    for kc in range(CH):
                        for nb in range(CH):
                            mm(psp[:, 8 * nb:8 * (nb + 1)],
                               wdec[:, 1 * 2048 + kc * 512 + nb * 128:1 * 2048 + kc * 512 + (nb + 1) * 128],
                               hbuf0[1][:, (T - 1) * 64 + 32 * g + 8 * kc:(T - 1) * 64 + 32 * g + 8 * kc + 8],
                               False, False)
                    for kc in range(CH):
                        for nb in range(CH):
                            mm(psp[:, 8 * nb:8 * (nb + 1)],
                               wdec[:, 4 * 2048 + kc * 512 + nb * 128:4 * 2048 + kc * 512 + (nb + 1) * 128],
                               pre_h[(0, g)][:, 8 * kc:8 * kc + 8],
                               False, kc == CH - 1 and nb == CH - 1)
                    h1p = hpool.tile([128, 32], F16, tag=f"hd1_{g}", name=f"hd1p{g}", bufs=2)
                    nc.scalar.activation(h1p[:], psp[:, 0:32], Tanh)
                    pre_h[(1, g)] = h1p

        # ---- decoder: 4-layer stack, 32 autoregressive steps ----
        # Split into two independent half-batch chains (8 rows each): the
        # halves' hops interleave like the encoder directions, halving both
        # the activation size and the serial matmul window per hop.
        DWIHR = 4 * 2048
        AOFF = 7 * 2048
        ph = None
        outt = hpool.tile([1, TGT * BC], F32, tag="out", name="out")

        def _e0_src(tile_, base):  # hbuf0 half-major layout
            return lambda g, kc: tile_[:, base + 32 * g + 8 * kc:base + 32 * g + 8 * kc + 8]

        def _enc_src(tile_, base):  # e1h batch-major layout
            return lambda g, kc: tile_[:, base + 16 * kc + 8 * g:base + 16 * kc + 8 * g + 8]

        hsrc = {0: lambda g, kc: pre_h[(0, g)][:, 8 * kc:8 * kc + 8],
                1: lambda g, kc: pre_h[(1, g)][:, 8 * kc:8 * kc + 8],
                2: _enc_src(e1h[0], 0), 3: _enc_src(e1h[1], 0)}

        def h3c(t, g, kc):
            return hbuf3[:, t * 64 + 32 * g + 8 * kc:t * 64 + 32 * g + 8 * kc + 8]

        xqf = lambda g: smalls[0:2, XQ + 8 * g:XQ + 8 * g + 8]
        ones8 = smalls[0:1, ONES:ONES + 8]
        for t in range(TGT):
            lays = range(2, 4) if t == 0 else range(4)  # l0/l1 at t=0 pre-ran in E1
            pss = {}
            for l in lays:
                for g in range(2):
                    pss[(l, g)] = pspool.tile(
                        [128, 512], F32, tag=f"ps0{g}",
                        name=f"psD{l}_{g}", bufs=2)
            # ready work: layer 0 xin terms + all layers' bias/Whh
            if t > 0:
                for g in range(2):
                    ps = pss[(0, g)]
                    for nb in range(CH):
                        mm(ps[:, 8 * nb:8 * (nb + 1)],
                           smalls[0:2, B2C + nb * 128:B2C + (nb + 1) * 128],
                           xqf(g), nb == 0, False)
                        mm(ps[:, 8 * nb:8 * (nb + 1)],
                           smalls[0:1, C0 + nb * 128:C0 + (nb + 1) * 128],
                           ones8, False, False)
                    for kc in range(CH):
                        for nb in range(CH):
                            mm(ps[:, 8 * nb:8 * (nb + 1)],
                               wdec[:, kc * 512 + nb * 128:kc * 512 + (nb + 1) * 128],
                               hsrc[0](g, kc), False, False)
            # layer 0 chain input (A @ h3_{t-1}) issued BEFORE the l1-3 ready
            # bursts: when act3_{t-1} fires, A is at the wait-queue head
            # instead of behind ~150 ready matmul pairs.
            if t > 0:
                for g in range(2):
                    ps = pss[(0, g)]
                    for kc in range(CH):
                        for nb in range(CH):
                            mm(ps[:, 8 * nb:8 * (nb + 1)],
                               wdec[:, AOFF + kc * 512 + nb * 128:AOFF + kc * 512 + (nb + 1) * 128],
                               h3c(t - 1, g, kc),
                               False, kc == CH - 1 and nb == CH - 1)
            for l in range(2 if t == 0 else 1, 4):
                for g in range(2):
                    ps = pss[(l, g)]
                    for nb in range(CH):
                        mm(ps[:, 8 * nb:8 * (nb + 1)],
                           smalls[0:1, DB + (l - 1) * 512 + nb * 128:DB + (l - 1) * 512 + (nb + 1) * 128],
                           ones8, nb == 0, False)
                    for kc in range(CH):
                        for nb in range(CH):
                            mm(ps[:, 8 * nb:8 * (nb + 1)],
                               wdec[:, l * 2048 + kc * 512 + nb * 128:l * 2048 + kc * 512 + (nb + 1) * 128],
                               hsrc[l](g, kc), False, False)
            if t > 0:
                ht0 = {}
                for g in range(2):
                    h0 = hpool.tile([128, 32], F16, tag=f"hd0_{g}", name=f"hd0_{g}", bufs=2)
                    nc.scalar.activation(h0[:], pss[(0, g)][:, 0:32], Tanh)
                    ht0[g] = h0
                hsrc[0] = lambda g, kc, _h=ht0: _h[g][:, 8 * kc:8 * kc + 8]
            # px/head fill the act0 wait (same trigger: act3_{t-1})
            if 1 <= t < TGT - 1:
                px = pspool.tile([128, 512], F32, tag="ps10", name="psX", bufs=2)
                for g in range(2):
                    for kc in range(CH):
                        mm(px[0:2, 8 * g:8 * g + 8],
                           smalls[:, WX2 + 2 * kc:WX2 + 2 * (kc + 1)],
                           h3c(t - 1, g, kc), g == 0 and kc == 0, False)
                    mm(px[0:2, 8 * g:8 * g + 8], smalls[0:2, S2C:S2C + 2],
                       xqf(g), False, False)
                    mm(px[0:2, 8 * g:8 * g + 8], smalls[0:1, CS:CS + 2],
                       ones8, False, g == 1)
                xnew = hpool.tile([2, 16], F16, tag="xin", name="xin", bufs=2)
                nc.vector.tensor_copy(xnew[:], px[0:2, 0:16])
            if t >= 1:
                # head rows stream in 8-step chunks, each its own group on an
                # alternating ph slot, evacuated + DMA'd as soon as complete
                # so only the final chunk sits after the last decoder step.
                if (t - 1) % 8 == 0:
                    ph = pspool.tile([128, 512], F32, tag="ps11", name="psH", bufs=2)
                for g in range(2):
                    for kc in range(CH):
                        mm(ph[0:1, 16 * (t - 1) + 8 * g:16 * (t - 1) + 8 * g + 8],
                           smalls[:, LINC + kc:LINC + kc + 1],
                           h3c(t - 1, g, kc), (t - 1) % 8 == 0 and g == 0 and kc == 0,
                           (t - 1) % 8 == 7 and g == 1 and kc == CH - 1)
                if (t - 1) % 8 == 7:
                    c = (t - 1) // 8
                    nc.scalar.activation(outt[0:1, 128 * c:128 * (c + 1)],
                                         ph[0:1, 128 * c:128 * (c + 1)],
                                         mybir.ActivationFunctionType.Identity,
                                         bias=smalls[0:1, LB:LB + 1])
                    nc.sync.dma_start(out_e[:, 128 * c:128 * (c + 1)],
                                      outt[0:1, 128 * c:128 * (c + 1)])
            # same-step Wih chains
            for l in range(2 if t == 0 else 1, 4):
                htl = {}
                for g in range(2):
                    ps = pss[(l, g)]
                    for kc in range(CH):
                        for nb in range(CH):
                            mm(ps[:, 8 * nb:8 * (nb + 1)],
                               wdec[:, DWIHR + (l - 1) * 2048 + kc * 512 + nb * 128:DWIHR + (l - 1) * 2048 + kc * 512 + (nb + 1) * 128],
                               hsrc[l - 1](g, kc),
                               False, kc == CH - 1 and nb == CH - 1)
                for g in range(2):
                    if l == 3:
                        nc.scalar.activation(hbuf3[:, t * 64 + 32 * g:t * 64 + 32 * (g + 1)],
                                             pss[(l, g)][:, 0:32], Tanh)
                    else:
                        hl = hpool.tile([128, 32], F16, tag=f"hd{l}_{g}", name=f"hd{l}_{g}", bufs=2)
                        nc.scalar.activation(hl[:], pss[(l, g)][:, 0:32], Tanh)
                        htl[g] = hl
                if l == 3:
                    hsrc[3] = lambda g, kc, _t=t: h3c(_t, g, kc)
                else:
                    hsrc[l] = lambda g, kc, _h=dict(htl): _h[g][:, 8 * kc:8 * kc + 8]
            if 1 <= t < TGT - 1:
                xqf = lambda g, _x=xnew: _x[0:2, 8 * g:8 * g + 8]
        for g in range(2):
            for kc in range(CH):  # head row for the final step (chunk 3 stop)
                mm(ph[0:1, 16 * (TGT - 1) + 8 * g:16 * (TGT - 1) + 8 * g + 8],
                   smalls[:, LINC + kc:LINC + kc + 1],
                   h3c(TGT - 1, g, kc), False, g == 1 and kc == CH - 1)
        nc.scalar.activation(outt[0:1, 384:512], ph[0:1, 384:512],
                             mybir.ActivationFunctionType.Identity,
                             bias=smalls[0:1, LB:LB + 1])
        nc.sync.dma_start(out_e[:, 384:512], outt[0:1, 384:512])

    nc.compile()
    _prog_cache["nc"] = nc
    return nc


def _statT(W):
    """W (N,K), h_new = W @ h -> stationary tile [128, (K//128)*N]:
    chunk kc at cols [kc*N:(kc+1)*N] holds W.T[128*kc:128*(kc+1), :]."""
    W = np.asarray(W, np.float32)
    N, K = W.shape
    WT = np.ascontiguousarray(W.T)
    return WT.reshape(K // 128, 128, N).transpose(1, 0, 2).reshape(128, (K // 128) * N)


def kernel(x, y, enc_Wih0, enc_Whh0, enc_Wih1, enc_Whh1, enc_bih, enc_bhh,
           dec_Wih0, dec_Wihr, dec_Whh, dec_bih, dec_bhh, lin_W, lin_b,
           target_len, teacher_forcing_ratio):
    f, h16 = np.float32, np.float16
    x = np.asarray(x, f)
    enc_Wih0, enc_Whh0 = np.asarray(enc_Wih0, f), np.asarray(enc_Whh0, f)
    enc_Wih1, enc_Whh1 = np.asarray(enc_Wih1, f), np.asarray(enc_Whh1, f)
    enc_bih, enc_bhh = np.asarray(enc_bih, f), np.asarray(enc_bhh, f)
    dec_Wih0, dec_Wihr = np.asarray(dec_Wih0, f), np.asarray(dec_Wihr, f)
    dec_Whh = np.asarray(dec_Whh, f)
    dec_bih, dec_bhh = np.asarray(dec_bih, f), np.asarray(dec_bhh, f)
    lin_W = np.asarray(lin_W, f)
    lb = float(np.asarray(lin_b, f).reshape(()))

    def _hilo(W):
        hi = W.astype(h16).astype(f)
        return [_statT(hi), _statT(W - hi)]

    whh0 = np.concatenate(_hilo(enc_Whh0[0]) + _hilo(enc_Whh0[1]), 1).astype(h16)
    wenc1 = np.concatenate([_statT(enc_Whh1[d]) for d in range(2)]
                           + [_statT(enc_Wih1[d]) for d in range(2)], 1).astype(h16)

    W0, linv = dec_Wih0, lin_W[0]  # (512,3), (512,)
    Nv = np.array([1.0, -1.0, 1.0], f)
    A = np.outer(W0 @ Nv, linv)                      # (512,512)
    b0tot = dec_bih[0] + dec_bhh[0]
    c0 = (W0 @ Nv) * lb + b0tot                      # (512,)
    B2 = np.stack([W0[:, 1] - W0[:, 2], W0[:, 2]])   # (2,512): x0,x1 coeffs
    wdec = np.concatenate([_statT(dec_Whh[l]) for l in range(4)]
                          + [_statT(dec_Wihr[l]) for l in range(3)]
                          + [_statT(A)], 1).astype(h16)

    smalls = np.zeros((128, C_SMALL), f)
    for d in range(2):
        smalls[0, d * 512:(d + 1) * 512] = enc_bih[1, d] + enc_bhh[1, d]
    for l in range(1, 4):
        smalls[0, DB + (l - 1) * 512:DB + l * 512] = dec_bih[l] + dec_bhh[l]
    smalls[0, C0:C0 + 512] = c0
    smalls[0, CS:CS + 2] = [lb, -lb]
    smalls[0:2, B2C:B2C + 512] = B2
    smalls[0:2, S2C:S2C + 2] = np.array([[0, 1], [0, 0]], f)
    din0q = np.zeros((4, 512), f)   # rows match xq rows (x0, x1, 1, x2)
    din0q[0], din0q[1], din0q[3] = W0[:, 0], W0[:, 1], W0[:, 2]
    din0q[2] = b0tot
    smalls[0:4, DIN0:DIN0 + 512] = din0q
    smalls[:, LINC:LINC + 4] = linv.reshape(4, 128).T
    wx2 = np.stack([linv, -linv])                    # (2,512)
    smalls[:, WX2:WX2 + 8] = wx2.T.reshape(4, 128, 2).transpose(1, 0, 2).reshape(128, 8)
    smalls[:, ONES:ONES + 16] = 1.0
    smalls[:, IDC:IDC + 128] = np.eye(128, dtype=f)
    smalls[0, LB] = lb

    nc = _build_program()

    in_maps = []
    for c in range(NC):
        xc = x[c * BC:(c + 1) * BC]  # (16, T, 3)
        xs0 = np.empty((2, 128, T * 4 * BC), h16)
        for d in range(2):
            W = enc_Wih0[d]
            b = enc_bih[0, d] + enc_bhh[0, d]
            proj = np.einsum('bti,hi->tbh', xc, W) + b   # (T, 16, 512)
            if d == 1:
                proj = proj[::-1]
            # half-major columns: t*64 + g*32 + kc*8 + j  (batch b = 8g+j)
            xs0[d] = proj.reshape(T, 2, BC // 2, 4, 128).transpose(4, 0, 1, 3, 2).reshape(128, T * 4 * BC)
        sm = smalls.copy()
        sm[0:2, XQ:XQ + 16] = xc[:, -1, 0:2].T
        sm[2, XQ:XQ + 16] = 1.0
        sm[3, XQ:XQ + 16] = xc[:, -1, 2]
        in_maps.append({
            "ident": np.eye(128, dtype=h16),
            "whh0": whh0, "xs0": xs0, "wenc1": wenc1, "wdec": wdec,
            "smalls": sm.astype(h16),
        })
    res = run_bass_kernel_spmd(nc, in_maps, list(range(NC)))
    out = np.stack([res.results[c]["out"].reshape(TGT, BC).T for c in range(NC)])
    return out.reshape(B, TGT, 1).astype(np.float32)
